# revision 1
# baseline (speedup 1.0000x reference)
"""Trainium2 Bass kernel for nn_EventTempRel_HGRU_static (hyperbolic GRU).

Strategy (pure data parallel over batch, 8 cores x 32 rows):
  * Host: fold weights (transposes, W.T@b columns, the cs_emb->W_ff_common
    mobius table), shard inputs.
  * Device phase 1 (precompute, pipelined with the scan): P_g = x @ U_g.T for
    all tokens via PE (fp32), plus per-token scalars (pp=|m0|^2, tau, q, pb).
    Uses the identity mobius_matvec(U, expmap0(x)) == expmap0(x @ U.T), with
    expmap0 applied lazily through stored tau scalars.
  * Device phase 2 (scan): 128 sequential hyperbolic-GRU steps on [32,128]
    tiles. All norms/dots via fused tensor_tensor_reduce / ACT Square+accum;
    all artanh/tanh/sqrt coefficient functions reduced to even functions of
    squared norms and evaluated as deg-2 minimax polys in ACT-Square form
    (p*x+q)^2 + e; divisions via the 1-instruction approx reciprocal.
  * Device phase 3 (head): mask-position gathers via indirect DMA, feedforward
    mobius layer, hyperbolic MLR with asinh/tanh Taylor series (ranges are
    tiny), output [32,4] per core.
"""
import os
import numpy as np

F32 = np.float32

# deg-2 minimax fits: f(x) ~ (P*x+Q)^2 + E on the stated range (all sgn=+1)
TAU_MV = (0.3616372627630415, -0.460824836532626, 0.7876404160046405)    # tanh(sqrt(u))/sqrt(u), u in [0,0.032]
TAU_PW = (0.3575593089268115, -0.4659240032983483, 0.7829142723793893)   # u in [0,0.070]
TAU_PRE = (0.3457879813610502, -0.4806491635079507, 0.7689669798854355)  # u in [0,0.185]
PHI_ST = (0.4647741909888864, 0.3581349612622114, 0.8717410381896435)    # artanh(sqrt(s))/sqrt(s), s in [0,0.070]
PHI_LOG = (0.49913475137469127, 0.33018529682360925, 0.8910181465187239) # s in [0,0.190]
PHI_HEAD = (0.4769722816867955, 0.3481376015135419, 0.8788082074322899)  # s in [0,0.115]
PSI_G = (0.01980701895330672, -1.059545853653833, 0.8975165215601002)    # artanh(1-1e-5)/sqrt(s), s in [31,33.2]

B, T, DIN, H, DOUT, C = 256, 128, 768, 128, 64, 4
NC_N = 8
BL = B // NC_N           # 32 batch rows per core
PW = 396                 # P-record floats per token: 384 m0 | 3 pb | 3 tau | 3 pp | 3 q
EPS = 1e-5

_CACHE = {}


def _split_multiwait(nc):
    import concourse.mybir as mybir
    import bass_rust
    for fn in nc.m.functions:
        for blk in fn.blocks:
            newinsts = []
            changed = False
            for inst in blk.instructions:
                si = inst.sync_info
                waits = list(si.on_wait) if si and si.on_wait else []
                if len(waits) > 1:
                    changed = True
                    for k, w in enumerate(waits[:-1]):
                        ev = mybir.InstEventSemaphore(
                            name=f"{inst.name}-w{k}", engine=inst.engine,
                            ins=[], outs=[],
                            sync_info=bass_rust.SyncInfo(on_wait=[w], on_update=[]))
                        newinsts.append(ev)
                    inst.sync_info = bass_rust.SyncInfo(on_wait=[waits[-1]],
                                                        on_update=si.on_update)
                newinsts.append(inst)
            if changed:
                blk.instructions = newinsts


def _make_tc_class():
    from concourse.tile import TileContext
    import bass_rust
    from bass_rust import ScopedClock

    class SplitDrainTC(TileContext):
        # this walrus build rejects instructions with >2 sem waits; split the
        # tile tail-drain's waits across single-wait vector nops.
        def _drain_and_barrier(self, tick_clock, wait_clock):
            nop = self.nc.vector.engine_nop()
            wait_clock.add_sem_waits(nop.ins,
                                     ScopedClock({None: tick_clock.global_clock}))
            si = nop.ins.sync_info
            waits = list(si.on_wait) if si and si.on_wait else []
            if len(waits) > 1:
                nop.ins.sync_info = bass_rust.SyncInfo(on_wait=waits[:1],
                                                       on_update=si.on_update)
                for w in waits[1:]:
                    n2 = self.nc.vector.engine_nop()
                    n2.ins.sync_info = bass_rust.SyncInfo(on_wait=[w], on_update=[])
            self.nc.sync.drain()
            self.nc.all_engine_barrier()
            popped = self.nc._tile_sem_poison_stack.pop()
            assert popped is self._sem_poison
            self.nc.clear_and_free_semaphores(list(self.sems.allocated().values()))
            self.nc.all_engine_barrier()

    return SplitDrainTC


def _build_program():
    import concourse.bass as bass
    import concourse.mybir as mybir
    TileContext = _make_tc_class()

    AF = mybir.ActivationFunctionType
    AL = mybir.AluOpType
    f32 = mybir.dt.float32
    i32 = mybir.dt.int32

    nc = bass.Bass()
    host = _pending_host

    # ---------------- DRAM I/O ----------------
    xseq = nc.dram_tensor("xseq", [BL, T, DIN], f32, kind="ExternalInput")
    m1d = nc.dram_tensor("m1d", [BL, T], f32, kind="ExternalInput")
    m2d = nc.dram_tensor("m2d", [BL, T], f32, kind="ExternalInput")
    cidd = nc.dram_tensor("cidd", [BL, 1], i32, kind="ExternalInput")
    identd = nc.dram_tensor("identd", [128, 128], f32, kind="ExternalInput")
    wzrxd = nc.dram_tensor("wzrxd", [128, 258], f32, kind="ExternalInput")
    whxd = nc.dram_tensor("whxd", [128, 129], f32, kind="ExternalInput")
    utxd = nc.dram_tensor("utxd", [768, 387], f32, kind="ExternalInput")
    bzrd = nc.dram_tensor("bzrd", [BL, 256], f32, kind="ExternalInput")
    bhd = nc.dram_tensor("bhd", [BL, 128], f32, kind="ExternalInput")
    cb2d = nc.dram_tensor("cb2d", [BL, 2], f32, kind="ExternalInput")
    wfuvd = nc.dram_tensor("wfuvd", [128, 128], f32, kind="ExternalInput")
    ctabd = nc.dram_tensor("ctabd", [20, DOUT], f32, kind="ExternalInput")
    pmld = nc.dram_tensor("pmld", [BL, C * DOUT], f32, kind="ExternalInput")
    auld = nc.dram_tensor("auld", [BL, C * DOUT], f32, kind="ExternalInput")
    # per-class consts rows: [ppc | pac | naf | ompc]  ([32, 4*C])
    clcd = nc.dram_tensor("clcd", [BL, 4 * C], f32, kind="ExternalInput")
    bffd = nc.dram_tensor("bffd", [BL, DOUT], f32, kind="ExternalInput")
    bdhd = nc.dram_tensor("bdhd", [BL, DOUT], f32, kind="ExternalInput")
    scald = nc.dram_tensor("scald", [1, 8], f32, kind="ExternalInput")  # n2bff, K0, (spare)
    outd = nc.dram_tensor("out", [BL, C], f32, kind="ExternalOutput")

    from contextlib import ExitStack
    with TileContext(nc) as tc, ExitStack() as _es:
        sb = _es.enter_context(tc.tile_pool(name="sb", bufs=2))
        ps = _es.enter_context(tc.tile_pool(name="ps", bufs=2, space="PSUM"))
        dr = _es.enter_context(tc.tile_pool(name="dr", bufs=1, space="DRAM"))

        pstore = dr.tile([T, BL, PW], f32, tag="pstore", name="pstore")
        hist = dr.tile([T, BL, H], f32, tag="hist", name="hist")

        # ---------------- load constants ----------------
        def cload(dram, shape, tag):
            t = sb.tile(shape, f32, tag=tag, bufs=1, name=tag)
            nc.sync.dma_start(out=t[:], in_=dram[:])
            return t

        ident = cload(identd, [128, 128], "ident")
        wzrx = cload(wzrxd, [128, 258], "wzrx")
        whx = cload(whxd, [128, 129], "whx")
        utx = sb.tile([128, 6 * 387], f32, tag="utx", bufs=1, name="utx")
        nc.sync.dma_start(out=utx[:].rearrange("p (c n) -> p c n", c=6),
                          in_=utxd[:].rearrange("(c p) n -> p c n", p=128))
        bzr = cload(bzrd, [BL, 256], "bzr")
        bh = cload(bhd, [BL, 128], "bh")
        cb2 = cload(cb2d, [BL, 2], "cb2")
        wfuv = cload(wfuvd, [128, 128], "wfuv")
        pml = cload(pmld, [BL, C * DOUT], "pml")
        aul = cload(auld, [BL, C * DOUT], "aul")
        clc = cload(clcd, [BL, 4 * C], "clc")
        bffb = cload(bffd, [BL, DOUT], "bffb")
        bdh = cload(bdhd, [BL, DOUT], "bdh")
        scal = cload(scald, [1, 8], "scal")
        m1t = cload(m1d, [BL, T], "m1t")
        m2t_ = cload(m2d, [BL, T], "m2t_")
        cidt = sb.tile([BL, 1], i32, tag="cidt", bufs=1, name="cidt")
        nc.sync.dma_start(out=cidt[:], in_=cidd[:])

        # ---------------- op helpers ----------------
        def tsv(out, in0, s1, s2=None, o0=AL.mult, o1=AL.bypass):
            nc.vector.tensor_scalar(out, in0, s1, s2, o0, o1)

        def tsg(out, in0, s1, s2=None, o0=AL.mult, o1=AL.bypass):
            nc.gpsimd.tensor_scalar(out, in0, s1, s2, o0, o1)

        def ttv(out, a, b, op=AL.mult):
            nc.vector.tensor_tensor(out=out, in0=a, in1=b, op=op)

        def ttg(out, a, b, op=AL.mult):
            nc.gpsimd.tensor_tensor(out=out, in0=a, in1=b, op=op)

        def stv(out, in0, s, in1, o0=AL.mult, o1=AL.add):
            nc.vector.scalar_tensor_tensor(out=out, in0=in0, scalar=s, in1=in1, op0=o0, op1=o1)

        def stg(out, in0, s, in1, o0=AL.mult, o1=AL.add):
            # gpsimd lacks scalar_tensor_tensor on this walrus: 2-op fallback
            nc.gpsimd.tensor_scalar(out, in0, s, None, o0, AL.bypass)
            nc.gpsimd.tensor_tensor(out=out, in0=out, in1=in1, op=o1)

        def ttr(scr_, a, b, acc):
            # dot product: TT mult + free-dim reduce (walrus here lacks the
            # fused tensor_tensor_reduce)
            nc.vector.tensor_tensor(out=scr_, in0=a, in1=b, op=AL.mult)
            nc.vector.tensor_reduce(acc, scr_, axis=mybir.AxisListType.X, op=AL.add)

        _cbias = {}

        def cb(val, parts):
            v = float(val)
            if v not in _cbias:
                tname = f"cbias{len(_cbias)}"
                tcb = sb.tile([128, 1], f32, tag=tname, bufs=1, name=tname)
                nc.vector.memset(tcb[:], v)
                _cbias[v] = tcb
            return _cbias[v][0:parts, 0:1]

        def _b(bias, out):
            if isinstance(bias, (int, float)) and float(bias) not in (0.0, 1.0):
                return cb(bias, out.shape[0])
            return bias

        def asq(out, in_, scale=1.0, bias=0.0, acc=None):
            nc.scalar.activation(out, in_, AF.Square, bias=_b(bias, out), scale=scale,
                                 accum_out=acc)

        def aid(out, in_, scale=1.0, bias=0.0):
            nc.scalar.activation(out, in_, AF.Identity, bias=_b(bias, out), scale=scale)

        def acp(out, in_, scale=1.0):
            nc.scalar.activation(out, in_, AF.Copy, bias=0.0, scale=scale)

        def rcp(out, in_):
            nc.vector.reciprocal(out, in_)

        def st(shape, tag, bufs=3):
            return sb.tile(shape, f32, tag=tag, bufs=bufs, name=tag)

        # ---------------- precompute chunk ----------------
        def emit_chunk(k):
            xch = sb.tile([128, DIN], f32, tag="xch", bufs=2, name="xch")
            nc.sync.dma_start(out=xch[:],
                              in_=xseq[:, 4 * k:4 * k + 4, :].rearrange("b t d -> t b d"))
            pm = ps.tile([128, 387], f32, tag="pmm", bufs=1, name="pmm")
            for j in range(6):
                pt_ps = ps.tile([128, 128], f32, tag="ptr", bufs=2, name="ptr")
                nc.tensor.transpose(out=pt_ps[:], in_=xch[:, j * 128:(j + 1) * 128],
                                    identity=ident[:])
                xt = sb.tile([128, 128], f32, tag=f"xt{j}", bufs=2)
                if j % 2 == 0:
                    nc.scalar.copy(xt[:], pt_ps[:])
                else:
                    nc.vector.tensor_copy(xt[:], pt_ps[:])
                nc.tensor.matmul(out=pm[:], lhsT=xt[:], rhs=utx[:, j * 387:(j + 1) * 387],
                                 start=(j == 0), stop=(j == 5))
            ptile = sb.tile([128, PW], f32, tag="ptile", bufs=2, name="ptile")
            nc.scalar.copy(ptile[:, 0:387], pm[:, 0:387])       # m0 + pb
            for g in range(3):
                scr_ = sb.tile([128, 128], f32, tag="scrp", bufs=3, name="scrp")
                asq(scr_[:], pm[:, g * 128:(g + 1) * 128],
                    acc=ptile[:, 390 + g:391 + g])               # pp
            ytau = sb.tile([128, 3], f32, tag="ytau", bufs=2, name="ytau")
            asq(ytau[:], ptile[:, 390:393], scale=TAU_PRE[0], bias=TAU_PRE[1])
            tsv(ptile[:, 387:390], ytau[:], TAU_PRE[2], None, AL.add)   # tau
            tq_ = sb.tile([128, 3], f32, tag="tqpre", bufs=2, name="tqpre")
            ttv(tq_[:], ptile[:, 387:390], ptile[:, 390:393])    # tau*pp
            ttv(ptile[:, 393:396], tq_[:], ptile[:, 387:390])    # q = tau^2*pp
            nc.sync.dma_start(
                out=pstore[4 * k:4 * k + 4].rearrange("t b n -> (t b) n"),
                in_=ptile[:])

        # ---------------- scan state ----------------
        h = st([BL, H], "h", bufs=3)
        nc.vector.memset(h[:], 0.0)
        hT = st([128, BL], "hT", bufs=3)
        nc.vector.memset(hT[:], 0.0)
        nh2 = st([BL, 1], "nh2", bufs=3)
        nc.vector.memset(nh2[:], 0.0)
        Ah = st([BL, 1], "Ah", bufs=3)
        nc.vector.memset(Ah[:], 1.0)
        ah2 = st([BL, 1], "ah2", bufs=3)
        nc.vector.memset(ah2[:], 1.0)
        c2d = st([BL, 1], "c2d", bufs=3)
        nc.vector.memset(c2d[:], 1.0)

        def emit_step(t):
            nonlocal h, hT, nh2, Ah, ah2, c2d
            P = sb.tile([BL, PW], f32, tag="pin", bufs=4, name="pin")
            nc.sync.dma_start(out=P[:], in_=pstore[t])
            Pzr = P[:, 0:256]
            Pm0h = P[:, 256:384]
            pbzr = P[:, 384:386]; pbh = P[:, 386:387]
            tzr = P[:, 387:389]; th_ = P[:, 389:390]
            ppzr = P[:, 390:392]; pph = P[:, 392:393]
            qzr = P[:, 393:395]; qh_ = P[:, 395:396]

            # --- gate matmul + dots ---
            ps1 = ps.tile([BL, 258], f32, tag="pg", bufs=2, name="pg")
            nc.tensor.matmul(out=ps1[:], lhsT=hT[:], rhs=wzrx[:], start=True, stop=True)
            m2 = st([BL, 2], "m2")
            mp = st([BL, 2], "mp")
            for g in range(2):
                scr_ = st([BL, 128], "scr", bufs=12)
                asq(scr_[:], ps1[:, g * 128:(g + 1) * 128], acc=m2[:, g:g + 1])
            scrw = st([BL, 256], "scrw", bufs=4)
            nc.vector.tensor_tensor(out=scrw[:], in0=ps1[:, 0:256], in1=Pzr, op=AL.mult)
            nc.vector.tensor_reduce(mp[:], scrw[:].rearrange("b (g h) -> b g h", g=2),
                                    axis=mybir.AxisListType.X, op=AL.add)
            mb = ps1[:, 256:258]

            # --- tau_w -> Czr ([32,2] packed) ---
            u2t = st([BL, 2], "u2t")
            tsv(u2t[:], m2[:], ah2[:, 0:1])
            ysq = st([BL, 2], "ysq")
            asq(ysq[:], u2t[:], scale=TAU_MV[0], bias=TAU_MV[1])
            Czr = st([BL, 2], "Czr")
            tsv(Czr[:], ysq[:], TAU_MV[2], Ah[:, 0:1], AL.add, AL.mult)

            # --- mobius #1 scalars (x = Czr*M, y = tau*m0) ---
            Czr2 = st([BL, 2], "Czr2"); ttv(Czr2[:], Czr[:], Czr[:])
            x2 = st([BL, 2], "x2"); ttv(x2[:], Czr2[:], m2[:])
            tq = st([BL, 2], "tq"); ttv(tq[:], Czr[:], tzr)
            xy = st([BL, 2], "xy"); ttv(xy[:], tq[:], mp[:])
            w = st([BL, 2], "w"); tsg(w[:], xy[:], 2.0, 1.0, AL.mult, AL.add)
            c1 = st([BL, 2], "c1"); ttg(c1[:], w[:], qzr, AL.add)
            dn0 = st([BL, 2], "dn0"); ttg(dn0[:], x2[:], qzr)
            den = st([BL, 2], "den"); ttg(den[:], dn0[:], w[:], AL.add)
            rr = st([BL, 2], "rr"); rcp(rr[:], den[:])
            c1r = st([BL, 2], "c1r"); ttg(c1r[:], c1[:], rr[:])
            C1 = st([BL, 2], "C1"); ttg(C1[:], c1r[:], Czr[:])
            c2_ = st([BL, 2], "c2_"); tsg(c2_[:], x2[:], -1.0, 1.0, AL.mult, AL.add)
            c2r = st([BL, 2], "c2r"); ttg(c2r[:], c2_[:], rr[:])
            C2t = st([BL, 2], "C2t"); ttg(C2t[:], c2r[:], tzr)

            # --- o1 (materialized per gate) ---
            o1 = st([BL, 256], "o1")
            for g in range(2):
                t0 = st([BL, 128], f"t0{g}")
                tsg(t0[:], Pzr[:, g * 128:(g + 1) * 128], C2t[:, g:g + 1])
                stv(o1[:, g * 128:(g + 1) * 128], ps1[:, g * 128:(g + 1) * 128],
                    C1[:, g:g + 1], t0[:])

            # --- mobius #2 (bias) scalars ---
            x2p = st([BL, 2], "x2p")
            xyp = st([BL, 2], "xyp")
            for g in range(2):
                scr_ = st([BL, 128], "scr", bufs=12)
                asq(scr_[:], o1[:, g * 128:(g + 1) * 128], acc=x2p[:, g:g + 1])
            scrw2 = st([BL, 256], "scrw", bufs=4)
            nc.vector.tensor_tensor(out=scrw2[:], in0=o1[:], in1=bzr[:], op=AL.mult)
            nc.vector.tensor_reduce(xyp[:], scrw2[:].rearrange("b (g h) -> b g h", g=2),
                                    axis=mybir.AxisListType.X, op=AL.add)
            wp = st([BL, 2], "wp"); tsg(wp[:], xyp[:], 2.0, 1.0, AL.mult, AL.add)
            c1p = st([BL, 2], "c1p"); ttg(c1p[:], wp[:], cb2[:], AL.add)
            dnp = st([BL, 2], "dnp"); ttg(dnp[:], x2p[:], cb2[:])
            denp = st([BL, 2], "denp"); ttg(denp[:], dnp[:], wp[:], AL.add)
            rrp = st([BL, 2], "rrp"); rcp(rrp[:], denp[:])
            D1 = st([BL, 2], "D1"); ttg(D1[:], c1p[:], rrp[:])
            c2pp = st([BL, 2], "c2pp"); tsg(c2pp[:], x2p[:], -1.0, 1.0, AL.mult, AL.add)
            D2 = st([BL, 2], "D2"); ttg(D2[:], c2pp[:], rrp[:])

            # --- o2 ---
            o2 = st([BL, 256], "o2")
            for g in range(2):
                t1 = st([BL, 128], f"t1{g}")
                acp(t1[:], bzr[:, g * 128:(g + 1) * 128], scale=D2[:, g:g + 1])
                stg(o2[:, g * 128:(g + 1) * 128], o1[:, g * 128:(g + 1) * 128],
                    D1[:, g:g + 1], t1[:])

            # --- s2, phi, gate preact, sigmoid ---
            s2 = st([BL, 2], "s2")
            for g in range(2):
                scr_ = st([BL, 128], "scr", bufs=12)
                asq(scr_[:], o2[:, g * 128:(g + 1) * 128], acc=s2[:, g:g + 1])
            yphi = st([BL, 2], "yphi")
            asq(yphi[:], s2[:], scale=PHI_LOG[0], bias=PHI_LOG[1])
            Ao = st([BL, 2], "Ao")
            tsv(Ao[:], yphi[:], PHI_LOG[2], None, AL.add)
            lg = st([BL, 256], "lg")
            tsv(lg[:, 0:128], o2[:, 0:128], Ao[:, 0:1])
            tsv(lg[:, 128:256], o2[:, 128:256], Ao[:, 1:2])
            zr = st([BL, 256], "zr")
            nc.scalar.activation(zr[:], lg[:], AF.Sigmoid)
            z_ = zr[:, 0:128]; r_ = zr[:, 128:256]

            # --- rh = mobius_pointwise_mul(h, r) ---
            wx = st([BL, 128], "wx"); ttg(wx[:], h[:], r_)
            sr = st([BL, 1], "sr")
            scr_ = st([BL, 128], "scr", bufs=12)
            asq(scr_[:], r_, acc=sr[:])
            nwx = st([BL, 1], "nwx")
            scr2_ = st([BL, 128], "scr", bufs=12)
            asq(scr2_[:], wx[:], acc=nwx[:])
            ypsi = st([BL, 1], "ypsi"); asq(ypsi[:], sr[:], scale=PSI_G[0], bias=PSI_G[1])
            crh = st([BL, 1], "crh"); tsv(crh[:], ypsi[:], PSI_G[2], None, AL.add)
            crh2 = st([BL, 1], "crh2"); asq(crh2[:], crh[:])
            u2 = st([BL, 1], "u2"); tsv(u2[:], crh2[:], nwx[:, 0:1])
            ytau2 = st([BL, 1], "ytau2"); asq(ytau2[:], u2[:], scale=TAU_PW[0], bias=TAU_PW[1])
            Crh = st([BL, 1], "Crh"); tsv(Crh[:], ytau2[:], TAU_PW[2], crh[:, 0:1], AL.add, AL.mult)
            Crh2 = st([BL, 1], "Crh2"); asq(Crh2[:], Crh[:])
            rh2 = st([BL, 1], "rh2"); tsv(rh2[:], Crh2[:], nwx[:, 0:1])
            yphi2 = st([BL, 1], "yphi2"); asq(yphi2[:], rh2[:], scale=PHI_ST[0], bias=PHI_ST[1])
            Arh = st([BL, 1], "Arh"); tsv(Arh[:], yphi2[:], PHI_ST[2], None, AL.add)
            arh2 = st([BL, 1], "arh2"); asq(arh2[:], Arh[:])

            # --- h-gate matmul on wx directly: rh@W = Crh*(wx@W), Crh folded
            # into the downstream coefficients (CzhC, arhc) so the transpose +
            # matmul only depend on wx, not on the psi/tau chain ---
            ptp = ps.tile([128, BL], f32, tag="ptp", bufs=1, name="ptp")
            nc.tensor.transpose(out=ptp[:], in_=wx[:], identity=ident[:BL, :BL])
            rhT = st([128, BL], "rhT")
            nc.scalar.copy(rhT[:], ptp[:])
            ps2 = ps.tile([BL, 129], f32, tag="ph", bufs=2, name="ph")
            nc.tensor.matmul(out=ps2[:], lhsT=rhT[:], rhs=whx[:], start=True, stop=True)
            m2h = st([BL, 1], "m2h")
            scr_ = st([BL, 128], "scr", bufs=12)
            asq(scr_[:], ps2[:, 0:128], acc=m2h[:])
            mph = st([BL, 1], "mph")
            scr2_ = st([BL, 128], "scr", bufs=12)
            ttr(scr2_[:], ps2[:, 0:128], Pm0h, mph[:])
            mbh = ps2[:, 128:129]

            arhc = st([BL, 1], "arhc"); acp(arhc[:], arh2[:], scale=Crh2[:, 0:1])
            uh = st([BL, 1], "uh"); tsv(uh[:], m2h[:], arhc[:, 0:1])
            ytauh = st([BL, 1], "ytauh"); asq(ytauh[:], uh[:], scale=TAU_MV[0], bias=TAU_MV[1])
            Czh0 = st([BL, 1], "Czh0"); tsv(Czh0[:], ytauh[:], TAU_MV[2], Arh[:, 0:1], AL.add, AL.mult)
            Czh = st([BL, 1], "Czh"); acp(Czh[:], Czh0[:], scale=Crh[:, 0:1])
            Czh2 = st([BL, 1], "Czh2"); asq(Czh2[:], Czh[:])
            x2h = st([BL, 1], "x2h"); tsv(x2h[:], Czh2[:], m2h[:, 0:1])
            tqh = st([BL, 1], "tqh"); acp(tqh[:], Czh[:], scale=th_)
            xyh = st([BL, 1], "xyh"); tsv(xyh[:], tqh[:], mph[:, 0:1])
            wh_ = st([BL, 1], "wh_"); aid(wh_[:], xyh[:], scale=2.0, bias=1.0)
            c1h = st([BL, 1], "c1h"); tsv(c1h[:], wh_[:], qh_, None, AL.add)
            denh = st([BL, 1], "denh"); tsv(denh[:], x2h[:], qh_, wh_[:, 0:1], AL.mult, AL.add)
            rrh = st([BL, 1], "rrh"); rcp(rrh[:], denh[:])
            c1rh = st([BL, 1], "c1rh"); acp(c1rh[:], c1h[:], scale=rrh[:, 0:1])
            C1h = st([BL, 1], "C1h"); acp(C1h[:], c1rh[:], scale=Czh[:, 0:1])
            c2h2 = st([BL, 1], "c2h2"); tsv(c2h2[:], x2h[:], -1.0, 1.0, AL.mult, AL.add)
            c2rh = st([BL, 1], "c2rh"); acp(c2rh[:], c2h2[:], scale=rrh[:, 0:1])
            C2th = st([BL, 1], "C2th"); acp(C2th[:], c2rh[:], scale=th_)

            tph = st([BL, 128], "tph"); tsg(tph[:], Pm0h, C2th[:, 0:1])
            o1h = st([BL, 128], "o1h")
            stv(o1h[:], ps2[:, 0:128], C1h[:, 0:1], tph[:])

            x2ph = st([BL, 1], "x2ph")
            scr_ = st([BL, 128], "scr", bufs=12)
            asq(scr_[:], o1h[:], acc=x2ph[:])
            xyph = st([BL, 1], "xyph")
            scr2_ = st([BL, 128], "scr", bufs=12)
            ttr(scr2_[:], o1h[:], bh[:], xyph[:])
            n2bh = host["n2bh"]
            wph = st([BL, 1], "wph"); aid(wph[:], xyph[:], scale=2.0, bias=1.0)
            c1ph = st([BL, 1], "c1ph"); tsv(c1ph[:], wph[:], n2bh, None, AL.add)
            denph = st([BL, 1], "denph"); tsv(denph[:], x2ph[:], n2bh, wph[:, 0:1], AL.mult, AL.add)
            rrph = st([BL, 1], "rrph"); rcp(rrph[:], denph[:])
            D1h = st([BL, 1], "D1h"); acp(D1h[:], c1ph[:], scale=rrph[:, 0:1])
            c2pph = st([BL, 1], "c2pph"); tsv(c2pph[:], x2ph[:], -1.0, 1.0, AL.mult, AL.add)
            D2h = st([BL, 1], "D2h"); acp(D2h[:], c2pph[:], scale=rrph[:, 0:1])
            G1 = st([BL, 1], "G1"); acp(G1[:], D1h[:], scale=C1h[:, 0:1])
            G2 = st([BL, 1], "G2"); acp(G2[:], D1h[:], scale=C2th[:, 0:1])

            tp2 = st([BL, 128], "tp2"); tsg(tp2[:], Pm0h, G2[:, 0:1])
            tm2 = st([BL, 128], "tm2")
            stv(tm2[:], ps2[:, 0:128], G1[:, 0:1], tp2[:])
            ht = st([BL, 128], "ht")
            stg(ht[:], bh[:], D2h[:, 0:1], tm2[:])

            # --- delta = mobius_add(-h, ht) ---
            y2d = st([BL, 1], "y2d")
            scr_ = st([BL, 128], "scr", bufs=12)
            asq(scr_[:], ht[:], acc=y2d[:])
            xyd = st([BL, 1], "xyd")
            scr2_ = st([BL, 128], "scr", bufs=12)
            ttr(scr2_[:], h[:], ht[:], xyd[:])
            wd = st([BL, 1], "wd"); aid(wd[:], xyd[:], scale=-2.0, bias=1.0)
            c1d = st([BL, 1], "c1d"); tsv(c1d[:], wd[:], y2d[:, 0:1], None, AL.add)
            dend = st([BL, 1], "dend"); tsv(dend[:], y2d[:], nh2[:, 0:1], wd[:, 0:1], AL.mult, AL.add)
            rrd = st([BL, 1], "rrd"); rcp(rrd[:], dend[:])
            Cd1 = st([BL, 1], "Cd1"); acp(Cd1[:], c1d[:], scale=rrd[:, 0:1])
            nCd1 = st([BL, 1], "nCd1"); tsv(nCd1[:], Cd1[:], -1.0)
            Cd2 = st([BL, 1], "Cd2"); acp(Cd2[:], c2d[:], scale=rrd[:, 0:1])
            td = st([BL, 128], "td"); acp(td[:], ht[:], scale=Cd2[:, 0:1])
            delta = st([BL, 128], "delta")
            stv(delta[:], h[:], nCd1[:, 0:1], td[:])

            # --- pw = pointwise(delta, z); h_new = mobius_add(h, pw) ---
            wx2 = st([BL, 128], "wx2"); ttg(wx2[:], delta[:], z_)
            sz = st([BL, 1], "sz")
            scr_ = st([BL, 128], "scr", bufs=12)
            asq(scr_[:], z_, acc=sz[:])
            nwx2 = st([BL, 1], "nwx2")
            scr2_ = st([BL, 128], "scr", bufs=12)
            asq(scr2_[:], wx2[:], acc=nwx2[:])
            xyp2 = st([BL, 1], "xyp2")
            scr3_ = st([BL, 128], "scr", bufs=12)
            ttr(scr3_[:], h[:], wx2[:], xyp2[:])
            ypsi2 = st([BL, 1], "ypsi2"); asq(ypsi2[:], sz[:], scale=PSI_G[0], bias=PSI_G[1])
            czp = st([BL, 1], "czp"); tsv(czp[:], ypsi2[:], PSI_G[2], None, AL.add)
            cz2p = st([BL, 1], "cz2p"); asq(cz2p[:], czp[:])
            u3 = st([BL, 1], "u3"); tsv(u3[:], cz2p[:], nwx2[:, 0:1])
            ytau3 = st([BL, 1], "ytau3"); asq(ytau3[:], u3[:], scale=TAU_PW[0], bias=TAU_PW[1])
            Cpw = st([BL, 1], "Cpw"); tsv(Cpw[:], ytau3[:], TAU_PW[2], czp[:, 0:1], AL.add, AL.mult)
            Cpw2 = st([BL, 1], "Cpw2"); asq(Cpw2[:], Cpw[:])
            y2n = st([BL, 1], "y2n"); tsv(y2n[:], Cpw2[:], nwx2[:, 0:1])
            xyn = st([BL, 1], "xyn"); tsv(xyn[:], Cpw[:], xyp2[:, 0:1])
            wn = st([BL, 1], "wn"); aid(wn[:], xyn[:], scale=2.0, bias=1.0)
            c1n = st([BL, 1], "c1n"); tsv(c1n[:], wn[:], y2n[:, 0:1], None, AL.add)
            denn = st([BL, 1], "denn"); tsv(denn[:], y2n[:], nh2[:, 0:1], wn[:, 0:1], AL.mult, AL.add)
            rrn = st([BL, 1], "rrn"); rcp(rrn[:], denn[:])
            C1n = st([BL, 1], "C1n"); acp(C1n[:], c1n[:], scale=rrn[:, 0:1])
            C2n = st([BL, 1], "C2n"); acp(C2n[:], c2d[:], scale=rrn[:, 0:1])
            C2nw = st([BL, 1], "C2nw"); tsv(C2nw[:], C2n[:], Cpw[:, 0:1])
            tn = st([BL, 128], "tn"); acp(tn[:], wx2[:], scale=C2nw[:, 0:1])
            h_new = st([BL, H], "h", bufs=3)
            stv(h_new[:], h[:], C1n[:, 0:1], tn[:])
            nc.sync.dma_start(out=hist[t], in_=h_new[:])

            # --- finalize state ---
            nh2n = st([BL, 1], "nh2", bufs=3)
            scr_ = st([BL, 128], "scr", bufs=12)
            asq(scr_[:], h_new[:], acc=nh2n[:])
            yphin = st([BL, 1], "yphin"); asq(yphin[:], nh2n[:], scale=PHI_ST[0], bias=PHI_ST[1])
            Ahn = st([BL, 1], "Ah", bufs=3); tsv(Ahn[:], yphin[:], PHI_ST[2], None, AL.add)
            ah2n = st([BL, 1], "ah2", bufs=3); asq(ah2n[:], Ahn[:])
            c2dn = st([BL, 1], "c2d", bufs=3); aid(c2dn[:], nh2n[:], scale=-1.0, bias=1.0)
            ptp2 = ps.tile([128, BL], f32, tag="ptp", bufs=1, name="ptp")
            nc.tensor.transpose(out=ptp2[:], in_=h_new[:], identity=ident[:BL, :BL])
            hTn = st([128, BL], "hT")
            nc.scalar.copy(hTn[:], ptp2[:])

            h, hT, nh2, Ah, ah2, c2d = h_new, hTn, nh2n, Ahn, ah2n, c2dn

        # ---------------- emit precompute + scan, pipelined ----------------
        for k in range(3):
            emit_chunk(k)
        for t in range(T):
            if t % 4 == 0 and (t // 4 + 3) < 32:
                emit_chunk(t // 4 + 3)
            emit_step(t)

        # ================= head =================
        iotaT = sb.tile([BL, T], i32, tag="iotaT", bufs=1)
        nc.gpsimd.iota(iotaT[:], pattern=[[1, T]], base=0, channel_multiplier=0)
        iotaTf = st([BL, T], "iotaTf", bufs=1)
        nc.vector.tensor_copy(iotaTf[:], iotaT[:])
        bidx = sb.tile([BL, 1], i32, tag="bidx", bufs=1, name="bidx")
        nc.gpsimd.iota(bidx[:], pattern=[[0, 1]], base=0, channel_multiplier=1)
        bidxf = st([BL, 1], "bidxf", bufs=1)
        nc.vector.tensor_copy(bidxf[:], bidx[:])

        uv = st([BL, 256], "uv", bufs=1)     # [u | v]
        for i, mt in enumerate((m1t, m2t_)):
            pos = st([BL, 1], f"pos{i}", bufs=1)
            scr_ = st([BL, 128], "scr", bufs=12)
            ttr(scr_[:], mt[:], iotaTf[:], pos[:])
            ridf = st([BL, 1], f"ridf{i}", bufs=1)
            stv(ridf[:], pos[:], float(BL), bidxf[:])
            ridi = sb.tile([BL, 1], i32, tag=f"ridi{i}", bufs=1)
            nc.vector.tensor_copy(ridi[:], ridf[:])
            nc.gpsimd.indirect_dma_start(
                out=uv[:, i * 128:(i + 1) * 128], out_offset=None,
                in_=hist[:].rearrange("t b h -> (t b) h"),
                in_offset=bass.IndirectOffsetOnAxis(ap=ridi[:, 0:1], axis=0))

        u_ap = uv[:, 0:128]; v_ap = uv[:, 128:256]

        # norms and cross dot
        x2u = st([BL, 1], "x2u", bufs=1)
        scr_ = st([BL, 128], "scr", bufs=12); asq(scr_[:], u_ap, acc=x2u[:])
        y2v = st([BL, 1], "y2v", bufs=1)
        scr_ = st([BL, 128], "scr", bufs=12); asq(scr_[:], v_ap, acc=y2v[:])
        xyuv = st([BL, 1], "xyuv", bufs=1)
        scr_ = st([BL, 128], "scr", bufs=12); ttr(scr_[:], u_ap, v_ap, xyuv[:])

        # dist coefficients (d = mobius_add(-u, v))
        wuv = st([BL, 1], "wuv", bufs=1); aid(wuv[:], xyuv[:], scale=-2.0, bias=1.0)
        c1uv = st([BL, 1], "c1uv", bufs=1); tsv(c1uv[:], wuv[:], y2v[:, 0:1], None, AL.add)
        denuv = st([BL, 1], "denuv", bufs=1); tsv(denuv[:], y2v[:], x2u[:, 0:1], wuv[:, 0:1], AL.mult, AL.add)
        ruv = st([BL, 1], "ruv", bufs=1); rcp(ruv[:], denuv[:])
        Cu1 = st([BL, 1], "Cu1", bufs=1); acp(Cu1[:], c1uv[:], scale=ruv[:, 0:1])
        Cu2t = st([BL, 1], "Cu2t", bufs=1); tsv(Cu2t[:], x2u[:], -1.0, 1.0, AL.mult, AL.add)
        Cu2 = st([BL, 1], "Cu2", bufs=1); acp(Cu2[:], Cu2t[:], scale=ruv[:, 0:1])
        # nd2 = Cu1^2 x2u - 2 Cu1 Cu2 xy + Cu2^2 y2v
        q1 = st([BL, 1], "q1", bufs=1); asq(q1[:], Cu1[:])
        q1x = st([BL, 1], "q1x", bufs=1); tsv(q1x[:], q1[:], x2u[:, 0:1])
        q2 = st([BL, 1], "q2", bufs=1); tsv(q2[:], Cu1[:], Cu2[:, 0:1])
        q2x = st([BL, 1], "q2x", bufs=1); tsv(q2x[:], q2[:], xyuv[:, 0:1])
        q3 = st([BL, 1], "q3", bufs=1); asq(q3[:], Cu2[:])
        q3x = st([BL, 1], "q3x", bufs=1); tsv(q3x[:], q3[:], y2v[:, 0:1])
        nd2 = st([BL, 1], "nd2", bufs=1)
        stv(nd2[:], q2x[:], -2.0, q1x[:])
        ttv(nd2[:], nd2[:], q3x[:], AL.add)
        # dsq*K0 = 2*K0*nd*phiH(nd2)   (Sqrt: the single table switch)
        nd = st([BL, 1], "nd", bufs=1)
        nc.scalar.activation(nd[:], nd2[:], AF.Sqrt)
        yph = st([BL, 1], "yph", bufs=1); asq(yph[:], nd2[:], scale=PHI_HEAD[0], bias=PHI_HEAD[1])
        phih = st([BL, 1], "phih", bufs=1); tsv(phih[:], yph[:], PHI_HEAD[2], None, AL.add)
        dsq0 = st([BL, 1], "dsq0", bufs=1); tsv(dsq0[:], nd[:], phih[:, 0:1])
        yk = st([BL, 1], "yk", bufs=1)
        tsv(yk[:], dsq0[:], host["k02"])
        # tanh(yk) series: yk*(1 - yk2/3 + 2/15 yk2^2)
        yk2 = st([BL, 1], "yk2", bufs=1); asq(yk2[:], yk[:])
        yk4 = st([BL, 1], "yk4", bufs=1); asq(yk4[:], yk2[:])
        tser = st([BL, 1], "tser", bufs=1)
        tsv(tser[:], yk2[:], float(-1.0 / 3.0), 1.0, AL.mult, AL.add)
        stv(tser[:], yk4[:], float(2.0 / 15.0), tser[:])
        tk = st([BL, 1], "tk", bufs=1); tsv(tk[:], tser[:], yk[:, 0:1])

        # FF matvecs: mu = u @ W_ff_u.T, mv = v @ W_ff_v.T
        ptu = ps.tile([128, BL], f32, tag="ptp", bufs=1, name="ptp")
        nc.tensor.transpose(out=ptu[:], in_=u_ap, identity=ident[:BL, :BL])
        uT = st([128, BL], "uT", bufs=1); nc.scalar.copy(uT[:], ptu[:])
        ptv = ps.tile([128, BL], f32, tag="ptp", bufs=1, name="ptp")
        nc.tensor.transpose(out=ptv[:], in_=v_ap, identity=ident[:BL, :BL])
        vT = st([128, BL], "vT", bufs=1); nc.scalar.copy(vT[:], ptv[:])
        psu = ps.tile([BL, DOUT], f32, tag="pg", bufs=2, name="pg")
        nc.tensor.matmul(out=psu[:], lhsT=uT[:], rhs=wfuv[:, 0:64], start=True, stop=True)
        psv = ps.tile([BL, DOUT], f32, tag="ph", bufs=2, name="ph")
        nc.tensor.matmul(out=psv[:], lhsT=vT[:], rhs=wfuv[:, 64:128], start=True, stop=True)

        def head_mvec(psx, sx, tag):
            # a = phi_ST(sx); n2 = |mx|^2; coef = a * tauMV(a^2*n2); out = coef*mx
            ya = st([BL, 1], f"ya{tag}", bufs=1); asq(ya[:], sx[:], scale=PHI_ST[0], bias=PHI_ST[1])
            a_ = st([BL, 1], f"a{tag}", bufs=1); tsv(a_[:], ya[:], PHI_ST[2], None, AL.add)
            n2_ = st([BL, 1], f"n2{tag}", bufs=1)
            scr_ = st([BL, 128], "scr", bufs=12)
            asq(scr_[:, 0:DOUT], psx[:], acc=n2_[:])
            a2_ = st([BL, 1], f"aa{tag}", bufs=1); asq(a2_[:], a_[:])
            uu_ = st([BL, 1], f"uu{tag}", bufs=1); tsv(uu_[:], a2_[:], n2_[:, 0:1])
            yt_ = st([BL, 1], f"yt{tag}", bufs=1); asq(yt_[:], uu_[:], scale=TAU_MV[0], bias=TAU_MV[1])
            cf_ = st([BL, 1], f"cf{tag}", bufs=1); tsv(cf_[:], yt_[:], TAU_MV[2], a_[:, 0:1], AL.add, AL.mult)
            mx = st([BL, DOUT], f"mx{tag}", bufs=1)
            tsv(mx[:], psx[:], cf_[:, 0:1])
            n2o = st([BL, 1], f"n2o{tag}", bufs=1)
            cf2 = st([BL, 1], f"cf2{tag}", bufs=1); asq(cf2[:], cf_[:])
            tsv(n2o[:], cf2[:], n2_[:, 0:1])
            return mx, n2o

        mu, n2mu = head_mvec(psu, x2u, "u")
        mv, n2mv = head_mvec(psv, y2v, "v")

        def head_mob_add(x_ap, x2_ap, y_ap, y2_ap, tag, y2_imm=None):
            """o = mobius_add(x, y) materialized [32,64]; returns (o, |o|^2)."""
            xy_ = st([BL, 1], f"hxy{tag}", bufs=1)
            scr_ = st([BL, 128], "scr", bufs=12)
            ttr(scr_[:, 0:DOUT], x_ap, y_ap, xy_[:])
            w_ = st([BL, 1], f"hw{tag}", bufs=1); aid(w_[:], xy_[:], scale=2.0, bias=1.0)
            c1_ = st([BL, 1], f"hc1{tag}", bufs=1)
            dn_ = st([BL, 1], f"hdn{tag}", bufs=1)
            if y2_imm is not None:
                tsv(c1_[:], w_[:], y2_imm, None, AL.add)
                tsv(dn_[:], x2_ap, y2_imm, w_[:, 0:1], AL.mult, AL.add)
            else:
                tsv(c1_[:], w_[:], y2_ap[:, 0:1], None, AL.add)
                tsv(dn_[:], x2_ap, y2_ap[:, 0:1], w_[:, 0:1], AL.mult, AL.add)
            rr_ = st([BL, 1], f"hrr{tag}", bufs=1); rcp(rr_[:], dn_[:])
            A1 = st([BL, 1], f"hA1{tag}", bufs=1); acp(A1[:], c1_[:], scale=rr_[:, 0:1])
            c2t_ = st([BL, 1], f"hc2{tag}", bufs=1); tsv(c2t_[:], x2_ap, -1.0, 1.0, AL.mult, AL.add)
            A2 = st([BL, 1], f"hA2{tag}", bufs=1); acp(A2[:], c2t_[:], scale=rr_[:, 0:1])
            t_ = st([BL, DOUT], f"hT{tag}", bufs=1)
            tsv(t_[:], y_ap, A2[:, 0:1])
            o_ = st([BL, DOUT], f"ho{tag}", bufs=1)
            stv(o_[:], x_ap, A1[:, 0:1], t_[:])
            so_ = st([BL, 1], f"hso{tag}", bufs=1)
            scr2_ = st([BL, 128], "scr", bufs=12)
            asq(scr2_[:, 0:DOUT], o_[:], acc=so_[:])
            return o_, so_

        o_, so_ = head_mob_add(mu[:], n2mu[:, 0:1], mv[:], n2mv, "a")
        o_, so_ = head_mob_add(o_[:], so_[:, 0:1], bffb[:], None, "b", y2_imm=host["n2bff"])
        # smul: sm = tk * bdh ; |sm|^2 = tk^2
        sm = st([BL, DOUT], "sm", bufs=1); tsv(sm[:], bdh[:], tk[:, 0:1])
        n2sm = st([BL, 1], "n2sm", bufs=1); asq(n2sm[:], tk[:])
        o_, so_ = head_mob_add(o_[:], so_[:, 0:1], sm[:], n2sm, "c")
        # common gather + add
        ct = st([BL, DOUT], "ct", bufs=1)
        nc.gpsimd.indirect_dma_start(
            out=ct[:], out_offset=None, in_=ctabd[:],
            in_offset=bass.IndirectOffsetOnAxis(ap=cidt[:, 0:1], axis=0))
        n2ct = st([BL, 1], "n2ct", bufs=1)
        scr_ = st([BL, 128], "scr", bufs=12)
        asq(scr_[:, 0:DOUT], ct[:], acc=n2ct[:])
        o_, so_ = head_mob_add(o_[:], so_[:, 0:1], ct[:], n2ct, "d")

        # ---- MLR: per class c ----
        pout = st([BL, C], "pout", bufs=1)
        aout = st([BL, C], "aout", bufs=1)
        for c in range(C):
            scr_ = st([BL, 128], "scr", bufs=12)
            ttr(scr_[:, 0:DOUT], o_[:], pml[:, c * DOUT:(c + 1) * DOUT], pout[:, c:c + 1])
            scr2_ = st([BL, 128], "scr", bufs=12)
            ttr(scr2_[:, 0:DOUT], o_[:], aul[:, c * DOUT:(c + 1) * DOUT], aout[:, c:c + 1])
        ppc = clc[:, 0:C]; pac = clc[:, C:2 * C]; naf = clc[:, 2 * C:3 * C]; ompc = clc[:, 3 * C:4 * C]
        # w = -2*pout + 1 ; c1 = w + so ; den = ppc*so + w
        wm = st([BL, C], "wm", bufs=1); aid(wm[:], pout[:], scale=-2.0, bias=1.0)
        c1m = st([BL, C], "c1m", bufs=1); tsv(c1m[:], wm[:], so_[:, 0:1], None, AL.add)
        dnm = st([BL, C], "dnm", bufs=1); tsv(dnm[:], ppc, so_[:, 0:1])
        ttv(dnm[:], dnm[:], wm[:], AL.add)
        rrm = st([BL, C], "rrm", bufs=1); rcp(rrm[:], dnm[:])
        C1m = st([BL, C], "C1m", bufs=1); ttv(C1m[:], c1m[:], rrm[:])
        C2m = st([BL, C], "C2m", bufs=1); ttv(C2m[:], ompc, rrm[:])
        # nmpx = C1^2 ppc - 2 C1 C2 pout + C2^2 so ; pda = -C1 pac + C2 aout
        s1m = st([BL, C], "s1m", bufs=1); asq(s1m[:], C1m[:]); ttv(s1m[:], s1m[:], ppc)
        s2m = st([BL, C], "s2m", bufs=1); ttv(s2m[:], C1m[:], C2m[:]); ttv(s2m[:], s2m[:], pout[:])
        s3m = st([BL, C], "s3m", bufs=1); asq(s3m[:], C2m[:]); tsv(s3m[:], s3m[:], so_[:, 0:1])
        nmpx = st([BL, C], "nmpx", bufs=1)
        stv(nmpx[:], s2m[:], -2.0, s1m[:])
        ttv(nmpx[:], nmpx[:], s3m[:], AL.add)
        pda = st([BL, C], "pda", bufs=1); ttv(pda[:], C1m[:], pac)
        pda2 = st([BL, C], "pda2", bufs=1); ttv(pda2[:], C2m[:], aout[:])
        ttv(pda[:], pda2[:], pda[:], AL.subtract)
        # lam = 2/(1-nmpx); arg = pda*lam
        omn = st([BL, C], "omn", bufs=1); aid(omn[:], nmpx[:], scale=-0.5, bias=0.5)
        lamr = st([BL, C], "lamr", bufs=1); rcp(lamr[:], omn[:])
        arg = st([BL, C], "arg", bufs=1); ttv(arg[:], pda[:], lamr[:])
        # asinh(arg) ~ arg*(1 - arg2/6 + 3/40 arg2^2)
        ag2 = st([BL, C], "ag2", bufs=1); asq(ag2[:], arg[:])
        ag4 = st([BL, C], "ag4", bufs=1); asq(ag4[:], ag2[:])
        gser = st([BL, C], "gser", bufs=1)
        tsv(gser[:], ag2[:], float(-1.0 / 6.0), 1.0, AL.mult, AL.add)
        stv(gser[:], ag4[:], float(3.0 / 40.0), gser[:])
        res = st([BL, C], "res", bufs=1)
        ttv(res[:], arg[:], gser[:])
        ttv(res[:], res[:], naf)
        nc.sync.dma_start(out=outd[:], in_=res[:])

    _split_multiwait(nc)
    return nc


def _host_constants(inputs):
    w_z = np.asarray(inputs['w_z'], F32); w_r = np.asarray(inputs['w_r'], F32)
    w_h = np.asarray(inputs['w_h'], F32)
    u_z = np.asarray(inputs['u_z'], F32); u_r = np.asarray(inputs['u_r'], F32)
    u_h = np.asarray(inputs['u_h'], F32)
    b_z = np.asarray(inputs['b_z'], F32); b_r = np.asarray(inputs['b_r'], F32)
    b_h = np.asarray(inputs['b_h'], F32)
    cs_emb = np.asarray(inputs['cs_emb'], F32)
    W_ff_u = np.asarray(inputs['W_ff_u'], F32); W_ff_v = np.asarray(inputs['W_ff_v'], F32)
    b_ff = np.asarray(inputs['b_ff'], F32); b_ff_d = np.asarray(inputs['b_ff_d'], F32)
    W_ff_common = np.asarray(inputs['W_ff_common'], F32)
    p_mlr = np.asarray(inputs['p_mlr'], F32); a_mlr = np.asarray(inputs['a_mlr'], F32)

    UT = np.concatenate([u_z, u_r, u_h], 0).T
    utb = np.stack([u_z.T @ b_z, u_r.T @ b_r, u_h.T @ b_h], 1)
    utx = np.ascontiguousarray(np.concatenate([UT, utb], 1), F32)       # [768,387]
    WzrT = np.concatenate([w_z, w_r], 0).T
    wtb = np.stack([w_z.T @ b_z, w_r.T @ b_r], 1)
    wzrx = np.ascontiguousarray(np.concatenate([WzrT, wtb], 1), F32)    # [128,258]
    whx = np.ascontiguousarray(
        np.concatenate([w_h.T, (w_h.T @ b_h)[:, None]], 1), F32)        # [128,129]

    bzr = np.ascontiguousarray(np.broadcast_to(
        np.concatenate([b_z, b_r])[None, :], (BL, 256)), F32)
    bhb = np.ascontiguousarray(np.broadcast_to(b_h[None, :], (BL, 128)), F32)
    cb2 = np.ascontiguousarray(np.broadcast_to(
        np.array([b_z @ b_z, b_r @ b_r], F32)[None, :], (BL, 2)), F32)

    wfuv = np.ascontiguousarray(np.concatenate([W_ff_u.T, W_ff_v.T], 1), F32)  # [128,128]

    # folded mobius_matvec(W_ff_common, cs_emb) table
    cs_n = np.linalg.norm(cs_emb, axis=1, keepdims=True).clip(1e-15)
    mcs = cs_emb @ W_ff_common.T
    mcs_n = np.linalg.norm(mcs, axis=1, keepdims=True).clip(1e-15)
    ctab = np.ascontiguousarray(
        np.tanh(mcs_n / cs_n * np.arctanh(np.clip(cs_n, None, 1.0 - EPS))) * mcs / mcs_n, F32)

    pml = np.ascontiguousarray(np.broadcast_to(
        p_mlr.reshape(1, C * DOUT), (BL, C * DOUT)), F32)
    na = np.linalg.norm(a_mlr.astype(np.float64), axis=-1)
    au = a_mlr / np.clip(na[:, None], 1e-12, None)
    aul = np.ascontiguousarray(np.broadcast_to(
        au.reshape(1, C * DOUT).astype(F32), (BL, C * DOUT)), F32)
    ppc = np.sum(p_mlr * p_mlr, -1)
    pac = np.sum(p_mlr * au, -1)
    clc = np.ascontiguousarray(np.broadcast_to(np.concatenate(
        [ppc, pac, 2.0 * na, 1.0 - ppc]).astype(F32)[None, :], (BL, 4 * C)), F32)

    bffb = np.ascontiguousarray(np.broadcast_to(b_ff[None, :], (BL, DOUT)), F32)
    nb = float(np.linalg.norm(b_ff_d.astype(np.float64)))
    K0 = float(np.arctanh(np.float32(min(nb, 1.0 - EPS))))
    bdh = np.ascontiguousarray(np.broadcast_to(
        (b_ff_d / F32(nb))[None, :], (BL, DOUT)), F32)
    scal = np.zeros((1, 8), F32)
    scal[0, 0] = b_ff @ b_ff
    scal[0, 1] = 2.0 * K0

    return dict(identd=np.eye(128, dtype=F32), wzrxd=wzrx, whxd=whx, utxd=utx,
                bzrd=bzr, bhd=bhb, cb2d=cb2, wfuvd=wfuv, ctabd=ctab, pmld=pml,
                auld=aul, clcd=clc, bffd=bffb, bdhd=bdh, scald=scal), \
        dict(n2bh=float(b_h @ b_h), n2bff=float(b_ff @ b_ff), k02=float(2.0 * K0))


LAST_RESULT = None


def kernel(**inputs):
    global LAST_RESULT
    from concourse.bass_utils import run_bass_kernel_spmd

    consts, scalars = _host_constants(inputs)

    import hashlib
    key = hashlib.sha1(repr(sorted(scalars.items())).encode()).hexdigest()
    if key not in _CACHE:
        # host[] scalars are baked as immediates during build
        _pending_host.clear()
        _pending_host.update(scalars)
        _CACHE[key] = _build_program()
    nc = _CACHE[key]

    seq = np.asarray(inputs['sequence'], F32)
    mask1 = np.ascontiguousarray(np.asarray(inputs['mask1'], F32)[:, :, 0])
    mask2 = np.ascontiguousarray(np.asarray(inputs['mask2'], F32)[:, :, 0])
    cids = np.asarray(inputs['common_ids']).astype(np.int32)

    in_maps = []
    for c in range(NC_N):
        sl = slice(c * BL, (c + 1) * BL)
        m = dict(consts)
        m['xseq'] = np.ascontiguousarray(seq[sl])
        m['m1d'] = np.ascontiguousarray(mask1[sl])
        m['m2d'] = np.ascontiguousarray(mask2[sl])
        m['cidd'] = np.ascontiguousarray(cids[sl])
        in_maps.append(m)

    res = run_bass_kernel_spmd(nc, in_maps, core_ids=list(range(NC_N)))
    LAST_RESULT = res
    out = np.concatenate([r['out'] for r in res.results], 0)
    return out


# host[] lookups inside _build_program resolve through this dict
_pending_host = {}


def _get_host():
    return _pending_host

